# revision 9
# baseline (speedup 1.0000x reference)
"""GCN (3-layer) on 8 TRN2 NeuronCores via Bass.

Strategy (graph/data parallel, per sharding hint):
 - Nodes are sharded 6250/core by destination id. Weights replicated.
 - Per layer L: each core computes its shard of the dense transform
   hs = dinv * (h @ W) (TensorE), shards are AllGather'd (bf16) into a
   full per-core gather table in DRAM.
 - Aggregation: edges sorted by destination; per 128-dest block, source
   rows are fetched with gpsimd dma_gather (int16 idx -> table split in
   lo/hi halves at 32768), segment-summed on TensorE via per-group
   one-hot selection matrices (built on DVE from iota + is_equal), and
   accumulated in PSUM. Epilogue scales by dinv[dest], adds bias, relu.
 - Layer L+1's GEMM streams per-block behind layer L's aggregation.
Output is produced transposed [64, 6250]/core and assembled on host.
"""

import sys

if "/opt/trn_rl_repo" not in sys.path:
    sys.path.insert(0, "/opt/trn_rl_repo")

import numpy as np
import ml_dtypes

P = 128
NCORE = 8
VLO = 32768  # lo/hi split of gather table (int16 index limit)

BF16 = ml_dtypes.bfloat16


# --------------------------------------------------------------------------
# host-side graph preprocessing (integer index work + array formatting only)
# --------------------------------------------------------------------------

def _wrap16(arr):
    """Gather index list [n] (n%16==0) -> [128, n//16] int16 (16-partition
    wrap, replicated for the 8 gpsimd cpus)."""
    n = len(arr)
    a = np.asarray(arr, dtype=np.int16).reshape(n // 16, 16).T
    return np.tile(a, (8, 1))


def _host_prep(edge_index, n_nodes, vlo=None):
    global VLO
    if vlo is not None:
        VLO = vlo
    sh = n_nodes // NCORE          # nodes per core
    nb = (sh + P - 1) // P         # dest blocks per core

    idx64 = np.asarray(edge_index).astype(np.int64)
    rows = np.concatenate([idx64[0], np.arange(n_nodes, dtype=np.int64)])
    cols = np.concatenate([idx64[1], np.arange(n_nodes, dtype=np.int64)])
    order = np.argsort(cols, kind="stable")
    rs = rows[order]
    cs = cols[order]
    deg = np.bincount(cols, minlength=n_nodes).astype(np.float32)

    # per (core, block) edge slices
    blk_edges = []  # [core][block] -> (lo_rows, hi_rows, lo_cr, hi_cr)
    nlo = np.zeros((NCORE, nb), np.int64)
    nhi = np.zeros((NCORE, nb), np.int64)
    for c in range(NCORE):
        per_core = []
        base = c * sh
        bounds = [min(base + P * b, base + sh) for b in range(nb + 1)]
        pos = np.searchsorted(cs, bounds)
        for b in range(nb):
            r = rs[pos[b]:pos[b + 1]]
            cr = cs[pos[b]:pos[b + 1]] - (base + P * b)
            lo_m = r < VLO
            per_core.append((r[lo_m], r[~lo_m] - VLO, cr[lo_m], cr[~lo_m]))
            nlo[c, b] = lo_m.sum()
            nhi[c, b] = len(r) - nlo[c, b]
        blk_edges.append(per_core)

    # shared (SPMD) schedule: per-block group counts = max over cores, min 1
    glo = [max(1, int(-(-nlo[:, b].max() // P))) for b in range(nb)]
    ghi = [max(1, int(-(-nhi[:, b].max() // P))) for b in range(nb)]
    maxg = max(glo[b] + ghi[b] for b in range(nb))
    totg = sum(glo) + sum(ghi)
    lo_off8 = np.concatenate([[0], np.cumsum([g * 8 for g in glo])]).astype(int)
    hi_off8 = np.concatenate([[0], np.cumsum([g * 8 for g in ghi])]).astype(int)
    grp_off = np.concatenate([[0], np.cumsum([glo[b] + ghi[b] for b in range(nb)])]).astype(int)

    per_core_inputs = []
    for c in range(NCORE):
        lo_parts, hi_parts, cr_parts = [], [], []
        for b in range(nb):
            lo_r, hi_r, lo_c, hi_c = blk_edges[c][b]
            lo_pad = glo[b] * P - len(lo_r)
            hi_pad = ghi[b] * P - len(hi_r)
            lo_parts.append(np.concatenate([lo_r, np.zeros(lo_pad, np.int64)]))
            hi_parts.append(np.concatenate([hi_r, np.zeros(hi_pad, np.int64)]))
            cr_parts.append(np.concatenate([
                lo_c, -np.ones(lo_pad, np.int64),
                hi_c, -np.ones(hi_pad, np.int64)]))
        idx_lo = _wrap16(np.concatenate(lo_parts))
        idx_hi = _wrap16(np.concatenate(hi_parts))
        crall = np.concatenate(cr_parts)
        colrel = np.ascontiguousarray(
            crall.reshape(totg, P).T.astype(np.float32).astype(BF16))
        deg_c = deg[c * sh:(c + 1) * sh]
        deg_bcast = np.ascontiguousarray(
            np.broadcast_to(deg_c[None, :], (P, sh)).astype(np.float32))
        deg_node = np.ascontiguousarray(
            np.concatenate([deg_c, np.ones(nb * P - sh, np.float32)])
            .reshape(nb, P).T.astype(np.float32))
        per_core_inputs.append(dict(
            idx_lo=np.ascontiguousarray(idx_lo),
            idx_hi=np.ascontiguousarray(idx_hi),
            colrel=colrel,
            deg_bcast=deg_bcast,
            deg_node=deg_node,
        ))

    sched = dict(sh=sh, nb=nb, glo=glo, ghi=ghi, maxg=maxg, totg=totg,
                 lo_off8=lo_off8, hi_off8=hi_off8, grp_off=grp_off,
                 loc=int(lo_off8[-1]), hic=int(hi_off8[-1]), n_nodes=n_nodes,
                 vlo=VLO)
    return sched, per_core_inputs


# --------------------------------------------------------------------------
# bass kernel builder
# --------------------------------------------------------------------------

def _build(sched):
    import concourse.bass as bass
    import concourse.bacc as bacc
    from concourse import mybir, library_config

    SH, NB = sched["sh"], sched["nb"]
    GLO, GHI = sched["glo"], sched["ghi"]
    MAXG, TOTG = sched["maxg"], sched["totg"]
    LO8, HI8, GOFF = sched["lo_off8"], sched["hi_off8"], sched["grp_off"]
    N = sched["n_nodes"]
    VLO_ = sched["vlo"]
    B = NB
    NL = 3                     # layers
    f32 = mybir.dt.float32
    bf16 = mybir.dt.bfloat16

    def wb_of(b):
        return min(P, SH - P * b)

    GCAP = int(__import__('os').environ.get('GCAP', '16'))  # max groups per dma_gather call
    ncalls = [(-(-GLO[b] // GCAP)) + (-(-GHI[b] // GCAP)) for b in range(B)]
    # g_cum[gidx] = required g_sems[gidx%3] threshold after block gidx's calls
    g_cum = []
    slot_tot = [0, 0, 0]
    for gi in range(NL * B):
        slot_tot[gi % 3] += 16 * ncalls[gi % B]
        g_cum.append(slot_tot[gi % 3])

    nc = bacc.Bacc(num_devices=NCORE)

    xT_d = nc.declare_dram_parameter("xT", [P, SH], f32, isOutput=False)
    degb_d = nc.declare_dram_parameter("deg_bcast", [P, SH], f32, isOutput=False)
    degn_d = nc.declare_dram_parameter("deg_node", [P, NB], f32, isOutput=False)
    idxlo_d = nc.declare_dram_parameter("idx_lo", [P, sched["loc"]], mybir.dt.int16, isOutput=False)
    idxhi_d = nc.declare_dram_parameter("idx_hi", [P, sched["hic"]], mybir.dt.int16, isOutput=False)
    colrel_d = nc.declare_dram_parameter("colrel", [P, TOTG], bf16, isOutput=False)
    w_d = [nc.declare_dram_parameter(f"W{i+1}", [P, P], f32, isOutput=False) for i in range(NL)]
    b_d = [nc.declare_dram_parameter(f"b{i+1}", [P, 1], f32, isOutput=False) for i in range(NL)]
    out_d = nc.declare_dram_parameter("out", [64, SH], f32, isOutput=True)

    bounce = [nc.dram_tensor(f"bounce{L}", [SH, P], bf16) for L in range(NL)]
    full = [nc.dram_tensor(f"full{L}", [N, P], bf16, addr_space="Shared")
            for L in range(NL)]

    NIN = 6 + 2 * NL  # input dma count

    from contextlib import ExitStack

    with ExitStack() as es:
        block = es.enter_context(nc.Block())
        xT_sb = es.enter_context(nc.sbuf_tensor("xT_sb", [P, SH], f32))
        hT_sb = es.enter_context(nc.sbuf_tensor("hT_sb", [P, SH], f32))
        dinv_dest = es.enter_context(nc.sbuf_tensor("dinv_dest", [P, SH], f32))
        dinv_node = es.enter_context(nc.sbuf_tensor("dinv_node", [P, NB], f32))
        idxlo_sb = es.enter_context(nc.sbuf_tensor("idxlo_sb", [P, sched["loc"]], mybir.dt.int16))
        idxhi_sb = es.enter_context(nc.sbuf_tensor("idxhi_sb", [P, sched["hic"]], mybir.dt.int16))
        colrel_sb = es.enter_context(nc.sbuf_tensor("colrel_sb", [P, TOTG], bf16))
        iota_i = es.enter_context(nc.sbuf_tensor("iota_i", [P, MAXG, P], mybir.dt.int32))
        iota_bf = es.enter_context(nc.sbuf_tensor("iota_bf", [P, MAXG, P], bf16))
        msg_sb = es.enter_context(nc.sbuf_tensor("msg_sb", [P, 3 * MAXG, P], bf16))
        s_sb = es.enter_context(nc.sbuf_tensor("s_sb", [P, 2, MAXG, P], bf16))
        epi_tmp = es.enter_context(nc.sbuf_tensor("epi_tmp", [P, 2, P], f32))
        stage_sb = es.enter_context(nc.sbuf_tensor("stage_sb", [P, 2, P], bf16))
        w_sb = es.enter_context(nc.sbuf_tensor("w_sb", [P, NL, P], f32))
        bias_sb = es.enter_context(nc.sbuf_tensor("bias_sb", [P, NL], f32))
        ps_a = [es.enter_context(nc.psum_tensor(f"ps_a{i}", [P, 512], f32)) for i in range(4)]
        ps_g = [es.enter_context(nc.psum_tensor(f"ps_g{i}", [P, 512], f32)) for i in range(2)]
        in_sem = es.enter_context(nc.semaphore("in_sem"))
        pool_sem = es.enter_context(nc.semaphore("pool_sem"))
        init_sem = es.enter_context(nc.semaphore("init_sem"))
        g_sems = [es.enter_context(nc.semaphore(f"g_sem{i}")) for i in range(3)]
        s_sem = es.enter_context(nc.semaphore("s_sem"))
        mm_sem = es.enter_context(nc.semaphore("mm_sem"))
        epi_sem = es.enter_context(nc.semaphore("epi_sem"))
        vepi_sem = es.enter_context(nc.semaphore("vepi_sem"))
        gmm_sem = es.enter_context(nc.semaphore("gmm_sem"))
        stage_sem = es.enter_context(nc.semaphore("stage_sem"))
        hsdma_sem = es.enter_context(nc.semaphore("hsdma_sem"))
        cc_sem = es.enter_context(nc.semaphore("cc_sem"))
        out_sem = es.enter_context(nc.semaphore("out_sem"))

        @block.sync
        def _(sync: bass.BassEngine):
            for dst, src in [
                (xT_sb[:], xT_d[:]), (dinv_dest[:], degb_d[:]),
                (dinv_node[:], degn_d[:]), (idxlo_sb[:], idxlo_d[:]),
                (idxhi_sb[:], idxhi_d[:]), (colrel_sb[:], colrel_d[:]),
            ]:
                sync.dma_start(out=dst, in_=src).then_inc(in_sem, 16)
            for i in range(NL):
                sync.dma_start(out=w_sb[:, i, :], in_=w_d[i][:]).then_inc(in_sem, 16)
                sync.dma_start(out=bias_sb[:, i:i + 1], in_=b_d[i][:]).then_inc(in_sem, 16)
            # staging dmas: GEMM_L' stages -> bounce[L']
            for Lp in range(NL):
                for b in range(B):
                    sidx = Lp * B + b
                    wb = wb_of(b)
                    sync.wait_ge(stage_sem, sidx + 1)
                    sync.dma_start(
                        out=bounce[Lp][P * b:P * b + wb, :],
                        in_=stage_sb[0:wb, sidx % 2, :],
                    ).then_inc(hsdma_sem, 16)
            sync.wait_ge(epi_sem, NL * B)
            sync.dma_start(out=out_d[:], in_=hT_sb[0:64, :]).then_inc(out_sem, 16)
            sync.wait_ge(out_sem, 16)

        @block.gpsimd
        def _(gpsimd: bass.BassGpSimd):
            gpsimd.load_library(library_config.mlp)
            for o in range(0, MAXG, GCAP):
                k = min(GCAP, MAXG - o)
                gpsimd.iota(iota_i[:, o:o + k, :], pattern=[[0, k], [1, P]],
                            base=0, channel_multiplier=0).then_inc(pool_sem, 1)
            for L in range(NL):
                gpsimd.wait_ge(hsdma_sem, 16 * B * (L + 1))
                gpsimd.collective_compute(
                    "AllGather", mybir.AluOpType.bypass,
                    replica_groups=[list(range(NCORE))],
                    ins=[bounce[L][:]], outs=[full[L][:]],
                ).then_inc(cc_sem, 1)
                gpsimd.wait_ge(cc_sem, L + 1)
                for b in range(B):
                    gidx = L * B + b
                    slot = gidx % 3
                    if gidx >= 3:
                        gpsimd.wait_ge(mm_sem, gidx - 2)
                    off = 0
                    for table, ng, idx_sb, off8 in (
                        (full[L][0:min(VLO_, N), :], GLO[b], idxlo_sb, LO8[b]),
                        (full[L][VLO_:N, :], GHI[b], idxhi_sb, HI8[b]),
                    ):
                        done = 0
                        while done < ng:
                            k = min(GCAP, ng - done)
                            gpsimd.dma_gather(
                                msg_sb[:, slot * MAXG + off:slot * MAXG + off + k, :],
                                table,
                                idx_sb[:, off8 + done * 8:off8 + (done + k) * 8],
                                k * P, k * P, P,
                            ).then_inc(g_sems[slot], 16)
                            done += k
                            off += k

        @block.scalar
        def _(scalar: bass.BassScalarEngine):
            scalar.wait_ge(init_sem, 1)
            scalar.activation(out=dinv_dest[:], in_=dinv_dest[:],
                              func=mybir.ActivationFunctionType.Sqrt)
            scalar.activation(out=dinv_node[:], in_=dinv_node[:],
                              func=mybir.ActivationFunctionType.Sqrt
                              ).then_inc(init_sem, 1)
            for L in range(NL):
                func = (mybir.ActivationFunctionType.Relu if L < NL - 1
                        else mybir.ActivationFunctionType.Identity)
                for b in range(B):
                    gidx = L * B + b
                    wb = wb_of(b)
                    bo = P * b
                    scalar.wait_ge(vepi_sem, gidx + 1)
                    scalar.activation(
                        out=hT_sb[:, bo:bo + wb],
                        in_=epi_tmp[:, gidx % 2, 0:wb],
                        func=func,
                        bias=bias_sb[:, L:L + 1],
                        scale=1.0,
                    ).then_inc(epi_sem, 1)

        @block.vector
        def _(vector: bass.BassVectorEngine):
            vector.wait_ge(in_sem, 16 * NIN)
            vector.wait_ge(pool_sem, -(-MAXG // GCAP))
            vector.tensor_copy(out=iota_bf[:], in_=iota_i[:])
            vector.reciprocal(out=dinv_dest[:], in_=dinv_dest[:])
            vector.reciprocal(out=dinv_node[:], in_=dinv_node[:]).then_inc(init_sem, 1)
            vector.wait_ge(init_sem, 2)

            def build_s(L, b):
                gidx = L * B + b
                if gidx >= 2:
                    vector.wait_ge(mm_sem, gidx - 1)
                g = GLO[b] + GHI[b]
                go = GOFF[b]
                vector.tensor_tensor(
                    out=s_sb[:, gidx % 2, 0:g, :],
                    in0=colrel_sb[:, go:go + g].to_broadcast([P, g, P]),
                    in1=iota_bf[:, 0:g, :],
                    op=mybir.AluOpType.is_equal,
                ).then_inc(s_sem, 1)

            def stage(Lp, b):
                sidx = Lp * B + b
                wb = wb_of(b)
                vector.wait_ge(gmm_sem, sidx + 1)
                if sidx >= 2:
                    vector.wait_ge(hsdma_sem, 16 * sidx)
                vector.tensor_scalar(
                    out=stage_sb[0:wb, sidx % 2, :],
                    in0=ps_g[sidx % 2][0:wb, 0:P],
                    scalar1=dinv_node[0:wb, b:b + 1],
                    scalar2=None,
                    op0=mybir.AluOpType.mult,
                ).then_inc(stage_sem, 1)

            for b in range(B):
                stage(0, b)
            for L in range(NL):
                build_s(L, 0)
                if B > 1:
                    build_s(L, 1)
                for b in range(B):
                    gidx = L * B + b
                    wb = wb_of(b)
                    bo = P * b
                    vector.wait_ge(mm_sem, gidx + 1)
                    if gidx >= 2:
                        vector.wait_ge(epi_sem, gidx - 1)
                    vector.tensor_tensor(
                        out=epi_tmp[:, gidx % 2, 0:wb],
                        in0=ps_a[gidx % 4][:, 0:wb],
                        in1=dinv_dest[:, bo:bo + wb],
                        op=mybir.AluOpType.mult,
                    ).then_inc(vepi_sem, 1)
                    if L < NL - 1:
                        stage(L + 1, b)
                    if b + 2 <= B - 1:
                        build_s(L, b + 2)

        @block.tensor
        def _(tensor: bass.BassTensorEngine):
            tensor.wait_ge(in_sem, 16 * NIN)

            def gemm(Lp, b, src_sb):
                sidx = Lp * B + b
                wb = wb_of(b)
                if Lp > 0:
                    tensor.wait_ge(epi_sem, (Lp - 1) * B + b + 1)
                if sidx >= 2:
                    tensor.wait_ge(stage_sem, sidx - 1)
                tensor.matmul(
                    out=ps_g[sidx % 2][0:wb, 0:P],
                    lhsT=src_sb[:, P * b:P * b + wb],
                    rhs=w_sb[:, Lp, :],
                    start=True, stop=True,
                ).then_inc(gmm_sem, 1)

            for b in range(B):
                gemm(0, b, xT_sb)
            for L in range(NL):
                for b in range(B):
                    gidx = L * B + b
                    slot = gidx % 3
                    tensor.wait_ge(g_sems[gidx % 3], g_cum[gidx])
                    tensor.wait_ge(s_sem, gidx + 1)
                    if gidx >= 4:
                        tensor.wait_ge(epi_sem, gidx - 3)
                    g_tot = GLO[b] + GHI[b]
                    for g in range(g_tot):
                        inst = tensor.matmul(
                            out=ps_a[gidx % 4][:, 0:P],
                            lhsT=msg_sb[:, slot * MAXG + g, :],
                            rhs=s_sb[:, gidx % 2, g, :],
                            start=(g == 0), stop=(g == g_tot - 1),
                        )
                    inst.then_inc(mm_sem, 1)
                    if L < NL - 1 and b >= 1:
                        gemm(L + 1, b - 1, hT_sb)
                if L < NL - 1:
                    gemm(L + 1, B - 1, hT_sb)

    nc.compile()
    return nc


# --------------------------------------------------------------------------
# entry point
# --------------------------------------------------------------------------

_CACHE = {}


def _get_compiled(edge_key, edge_index, n_nodes):
    if edge_key in _CACHE:
        return _CACHE[edge_key]
    sched, per_core = _host_prep(edge_index, n_nodes)
    nc = _build(sched)
    _CACHE[edge_key] = (sched, per_core, nc)
    return _CACHE[edge_key]


def run_device(x, edge_index, Ws, bs, trace=False):
    """Builds inputs, runs the SPMD kernel, returns (out [N,64], results)."""
    from concourse.bass_utils import run_bass_kernel_spmd

    n_nodes = x.shape[0]
    edge_key = hash(np.asarray(edge_index).tobytes()) ^ n_nodes
    sched, per_core, nc = _get_compiled(edge_key, edge_index, n_nodes)
    sh = sched["sh"]

    x = np.asarray(x, np.float32)
    xT = np.ascontiguousarray(x.T)
    W1, W2, W3 = (np.asarray(w, np.float32) for w in Ws)
    b1, b2, b3 = (np.asarray(b, np.float32).reshape(-1) for b in bs)
    W3p = np.zeros((P, P), np.float32)
    W3p[:, :W3.shape[1]] = W3
    b3p = np.zeros((P,), np.float32)
    b3p[:b3.shape[0]] = b3

    in_maps = []
    for c in range(NCORE):
        pc = per_core[c]
        in_maps.append({
            "xT": np.ascontiguousarray(xT[:, c * sh:(c + 1) * sh]),
            "deg_bcast": pc["deg_bcast"],
            "deg_node": pc["deg_node"],
            "idx_lo": pc["idx_lo"],
            "idx_hi": pc["idx_hi"],
            "colrel": pc["colrel"],
            "W1": W1, "W2": W2, "W3": W3p,
            "b1": np.ascontiguousarray(b1[:, None]),
            "b2": np.ascontiguousarray(b2[:, None]),
            "b3": np.ascontiguousarray(b3p[:, None]),
        })

    res = run_bass_kernel_spmd(nc, in_maps, core_ids=list(range(NCORE)),
                               trace=trace)
    outs = [res.results[c]["out"] for c in range(NCORE)]
    out = np.concatenate(outs, axis=1).T[:, :W3.shape[1]]
    return np.ascontiguousarray(out, dtype=np.float32), res


def kernel(x, edge_index, W1, b1, W2, b2, W3, b3):
    out, _ = run_device(x, edge_index, (W1, W2, W3), (b1, b2, b3))
    return out


# revision 10
# speedup vs baseline: 1.8780x; 1.8780x over previous
"""GCN (3-layer) on 8 TRN2 NeuronCores via Bass.

Strategy (graph/data parallel, per sharding hint):
 - Nodes are sharded 6250/core by destination id. Weights replicated.
 - Per layer L: each core computes its shard of the dense transform
   hs = dinv * (h @ W) (TensorE), shards are AllGather'd (bf16) into a
   full per-core gather table in DRAM.
 - Aggregation: edges sorted by destination; per 128-dest block, source
   rows are fetched with gpsimd dma_gather (int16 idx -> table split in
   lo/hi halves at 32768), segment-summed on TensorE via per-group
   one-hot selection matrices (built on DVE from iota + is_equal), and
   accumulated in PSUM. Epilogue scales by dinv[dest], adds bias, relu.
 - Layer L+1's GEMM streams per-block behind layer L's aggregation.
Output is produced transposed [64, 6250]/core and assembled on host.
"""

import sys

if "/opt/trn_rl_repo" not in sys.path:
    sys.path.insert(0, "/opt/trn_rl_repo")

import numpy as np
import ml_dtypes

P = 128
NCORE = 8
VLO = 32768  # lo/hi split of gather table (int16 index limit)

BF16 = ml_dtypes.bfloat16


# --------------------------------------------------------------------------
# host-side graph preprocessing (integer index work + array formatting only)
# --------------------------------------------------------------------------

def _wrap16(arr):
    """Gather index list [n] (n%16==0) -> [128, n//16] int16 (16-partition
    wrap, replicated for the 8 gpsimd cpus)."""
    n = len(arr)
    a = np.asarray(arr, dtype=np.int16).reshape(n // 16, 16).T
    return np.tile(a, (8, 1))


def _host_prep(edge_index, n_nodes, vlo=None):
    global VLO
    if vlo is not None:
        VLO = vlo
    sh = n_nodes // NCORE          # nodes per core
    nb = (sh + P - 1) // P         # dest blocks per core

    idx64 = np.asarray(edge_index).astype(np.int64)
    rows = np.concatenate([idx64[0], np.arange(n_nodes, dtype=np.int64)])
    cols = np.concatenate([idx64[1], np.arange(n_nodes, dtype=np.int64)])
    order = np.argsort(cols, kind="stable")
    rs = rows[order]
    cs = cols[order]
    deg = np.bincount(cols, minlength=n_nodes).astype(np.float32)

    # per (core, block) edge slices
    blk_edges = []  # [core][block] -> (lo_rows, hi_rows, lo_cr, hi_cr)
    nlo = np.zeros((NCORE, nb), np.int64)
    nhi = np.zeros((NCORE, nb), np.int64)
    for c in range(NCORE):
        per_core = []
        base = c * sh
        bounds = [min(base + P * b, base + sh) for b in range(nb + 1)]
        pos = np.searchsorted(cs, bounds)
        for b in range(nb):
            r = rs[pos[b]:pos[b + 1]]
            cr = cs[pos[b]:pos[b + 1]] - (base + P * b)
            lo_m = r < VLO
            per_core.append((r[lo_m], r[~lo_m] - VLO, cr[lo_m], cr[~lo_m]))
            nlo[c, b] = lo_m.sum()
            nhi[c, b] = len(r) - nlo[c, b]
        blk_edges.append(per_core)

    # shared (SPMD) schedule: per-block group counts = max over cores, min 1
    glo = [max(1, int(-(-nlo[:, b].max() // P))) for b in range(nb)]
    ghi = [max(1, int(-(-nhi[:, b].max() // P))) for b in range(nb)]
    maxg = max(glo[b] + ghi[b] for b in range(nb))
    totg = sum(glo) + sum(ghi)
    lo_off8 = np.concatenate([[0], np.cumsum([g * 8 for g in glo])]).astype(int)
    hi_off8 = np.concatenate([[0], np.cumsum([g * 8 for g in ghi])]).astype(int)
    grp_off = np.concatenate([[0], np.cumsum([glo[b] + ghi[b] for b in range(nb)])]).astype(int)

    per_core_inputs = []
    for c in range(NCORE):
        lo_parts, hi_parts, cr_parts = [], [], []
        for b in range(nb):
            lo_r, hi_r, lo_c, hi_c = blk_edges[c][b]
            lo_pad = glo[b] * P - len(lo_r)
            hi_pad = ghi[b] * P - len(hi_r)
            lo_parts.append(np.concatenate([lo_r, np.zeros(lo_pad, np.int64)]))
            hi_parts.append(np.concatenate([hi_r, np.zeros(hi_pad, np.int64)]))
            cr_parts.append(np.concatenate([
                lo_c, -np.ones(lo_pad, np.int64),
                hi_c, -np.ones(hi_pad, np.int64)]))
        idx_lo = _wrap16(np.concatenate(lo_parts))
        idx_hi = _wrap16(np.concatenate(hi_parts))
        crall = np.concatenate(cr_parts)
        colrel = np.ascontiguousarray(
            crall.reshape(totg, P).T.astype(np.float32).astype(BF16))
        deg_c = deg[c * sh:(c + 1) * sh]
        deg_bcast = np.ascontiguousarray(
            np.broadcast_to(deg_c[None, :], (P, sh)).astype(np.float32))
        deg_node = np.ascontiguousarray(
            np.concatenate([deg_c, np.ones(nb * P - sh, np.float32)])
            .reshape(nb, P).T.astype(np.float32))
        per_core_inputs.append(dict(
            idx_lo=np.ascontiguousarray(idx_lo),
            idx_hi=np.ascontiguousarray(idx_hi),
            colrel=colrel,
            deg_bcast=deg_bcast,
            deg_node=deg_node,
        ))

    sched = dict(sh=sh, nb=nb, glo=glo, ghi=ghi, maxg=maxg, totg=totg,
                 lo_off8=lo_off8, hi_off8=hi_off8, grp_off=grp_off,
                 loc=int(lo_off8[-1]), hic=int(hi_off8[-1]), n_nodes=n_nodes,
                 vlo=VLO)
    return sched, per_core_inputs


# --------------------------------------------------------------------------
# bass kernel builder
# --------------------------------------------------------------------------

def _build(sched):
    import concourse.bass as bass
    import concourse.bacc as bacc
    from concourse import mybir, library_config

    SH, NB = sched["sh"], sched["nb"]
    GLO, GHI = sched["glo"], sched["ghi"]
    MAXG, TOTG = sched["maxg"], sched["totg"]
    LO8, HI8, GOFF = sched["lo_off8"], sched["hi_off8"], sched["grp_off"]
    N = sched["n_nodes"]
    VLO_ = sched["vlo"]
    B = NB
    NL = 3                     # layers
    f32 = mybir.dt.float32
    bf16 = mybir.dt.bfloat16

    def wb_of(b):
        return min(P, SH - P * b)

    GCAP = int(__import__('os').environ.get('GCAP', '16'))  # max groups per dma_gather call
    ncalls = [(-(-GLO[b] // GCAP)) + (-(-GHI[b] // GCAP)) for b in range(B)]
    # g_cum[gidx] = required g_sems[gidx%3] threshold after block gidx's calls
    g_cum = []
    slot_tot = [0, 0, 0]
    for gi in range(NL * B):
        slot_tot[gi % 3] += 16 * ncalls[gi % B]
        g_cum.append(slot_tot[gi % 3])

    import os as _os
    NSWQ = int(_os.environ.get("NSWQ", "1"))
    nc = bacc.Bacc(num_devices=NCORE, num_swdge_queues=NSWQ)

    xT_d = nc.declare_dram_parameter("xT", [P, SH], f32, isOutput=False)
    degb_d = nc.declare_dram_parameter("deg_bcast", [P, SH], f32, isOutput=False)
    degn_d = nc.declare_dram_parameter("deg_node", [P, NB], f32, isOutput=False)
    idxlo_d = nc.declare_dram_parameter("idx_lo", [P, sched["loc"]], mybir.dt.int16, isOutput=False)
    idxhi_d = nc.declare_dram_parameter("idx_hi", [P, sched["hic"]], mybir.dt.int16, isOutput=False)
    colrel_d = nc.declare_dram_parameter("colrel", [P, TOTG], bf16, isOutput=False)
    w_d = [nc.declare_dram_parameter(f"W{i+1}", [P, P], f32, isOutput=False) for i in range(NL)]
    b_d = [nc.declare_dram_parameter(f"b{i+1}", [P, 1], f32, isOutput=False) for i in range(NL)]
    out_d = nc.declare_dram_parameter("out", [64, SH], f32, isOutput=True)

    bounce = [nc.dram_tensor(f"bounce{L}", [SH, P], bf16) for L in range(NL)]
    full = [nc.dram_tensor(f"full{L}", [N, P], bf16, addr_space="Shared")
            for L in range(NL)]

    NIN = 6 + 2 * NL  # input dma count

    from contextlib import ExitStack

    with ExitStack() as es:
        block = es.enter_context(nc.Block())
        xT_sb = es.enter_context(nc.sbuf_tensor("xT_sb", [P, SH], f32))
        hT_sb = es.enter_context(nc.sbuf_tensor("hT_sb", [P, SH], f32))
        dinv_dest = es.enter_context(nc.sbuf_tensor("dinv_dest", [P, SH], f32))
        dinv_node = es.enter_context(nc.sbuf_tensor("dinv_node", [P, NB], f32))
        idxlo_sb = es.enter_context(nc.sbuf_tensor("idxlo_sb", [P, sched["loc"]], mybir.dt.int16))
        idxhi_sb = es.enter_context(nc.sbuf_tensor("idxhi_sb", [P, sched["hic"]], mybir.dt.int16))
        colrel_sb = es.enter_context(nc.sbuf_tensor("colrel_sb", [P, TOTG], bf16))
        iota_i = es.enter_context(nc.sbuf_tensor("iota_i", [P, MAXG, P], mybir.dt.int32))
        iota_bf = es.enter_context(nc.sbuf_tensor("iota_bf", [P, MAXG, P], bf16))
        msg_sb = es.enter_context(nc.sbuf_tensor("msg_sb", [P, 3 * MAXG, P], bf16))
        s_sb = es.enter_context(nc.sbuf_tensor("s_sb", [P, 2, MAXG, P], bf16))
        epi_tmp = es.enter_context(nc.sbuf_tensor("epi_tmp", [P, 2, P], f32))
        stage_sb = es.enter_context(nc.sbuf_tensor("stage_sb", [P, 2, P], bf16))
        w_sb = es.enter_context(nc.sbuf_tensor("w_sb", [P, NL, P], f32))
        bias_sb = es.enter_context(nc.sbuf_tensor("bias_sb", [P, NL], f32))
        ps_a = [es.enter_context(nc.psum_tensor(f"ps_a{i}", [P, 512], f32)) for i in range(4)]
        ps_g = [es.enter_context(nc.psum_tensor(f"ps_g{i}", [P, 512], f32)) for i in range(2)]
        in_sem = es.enter_context(nc.semaphore("in_sem"))
        pool_sem = es.enter_context(nc.semaphore("pool_sem"))
        init_sem = es.enter_context(nc.semaphore("init_sem"))
        g_sems = [es.enter_context(nc.semaphore(f"g_sem{i}")) for i in range(3)]
        s_sem = es.enter_context(nc.semaphore("s_sem"))
        mm_sem = es.enter_context(nc.semaphore("mm_sem"))
        epi_sem = es.enter_context(nc.semaphore("epi_sem"))
        vepi_sem = es.enter_context(nc.semaphore("vepi_sem"))
        gmm_sem = es.enter_context(nc.semaphore("gmm_sem"))
        stage_sem = es.enter_context(nc.semaphore("stage_sem"))
        hsdma_sem = es.enter_context(nc.semaphore("hsdma_sem"))
        cc_sem = es.enter_context(nc.semaphore("cc_sem"))
        out_sem = es.enter_context(nc.semaphore("out_sem"))

        @block.sync
        def _(sync: bass.BassEngine):
            for dst, src in [
                (xT_sb[:], xT_d[:]), (dinv_dest[:], degb_d[:]),
                (dinv_node[:], degn_d[:]), (idxlo_sb[:], idxlo_d[:]),
                (idxhi_sb[:], idxhi_d[:]), (colrel_sb[:], colrel_d[:]),
            ]:
                sync.dma_start(out=dst, in_=src).then_inc(in_sem, 16)
            for i in range(NL):
                sync.dma_start(out=w_sb[:, i, :], in_=w_d[i][:]).then_inc(in_sem, 16)
                sync.dma_start(out=bias_sb[:, i:i + 1], in_=b_d[i][:]).then_inc(in_sem, 16)
            # staging dmas: GEMM_L' stages -> bounce[L']
            for Lp in range(NL):
                for b in range(B):
                    sidx = Lp * B + b
                    wb = wb_of(b)
                    sync.wait_ge(stage_sem, sidx + 1)
                    sync.dma_start(
                        out=bounce[Lp][P * b:P * b + wb, :],
                        in_=stage_sb[0:wb, sidx % 2, :],
                    ).then_inc(hsdma_sem, 16)
            sync.wait_ge(epi_sem, NL * B)
            sync.dma_start(out=out_d[:], in_=hT_sb[0:64, :]).then_inc(out_sem, 16)
            sync.wait_ge(out_sem, 16)

        qctr = [0]

        @block.gpsimd
        def _(gpsimd: bass.BassGpSimd):
            gpsimd.load_library(library_config.mlp)
            for o in range(0, MAXG, GCAP):
                k = min(GCAP, MAXG - o)
                gpsimd.iota(iota_i[:, o:o + k, :], pattern=[[0, k], [1, P]],
                            base=0, channel_multiplier=0).then_inc(pool_sem, 1)
            for L in range(NL):
                gpsimd.wait_ge(hsdma_sem, 16 * B * (L + 1))
                gpsimd.collective_compute(
                    "AllGather", mybir.AluOpType.bypass,
                    replica_groups=[list(range(NCORE))],
                    ins=[bounce[L][:]], outs=[full[L][:]],
                ).then_inc(cc_sem, 1)
                gpsimd.wait_ge(cc_sem, L + 1)
                for b in range(B):
                    gidx = L * B + b
                    slot = gidx % 3
                    if gidx >= 3:
                        gpsimd.wait_ge(mm_sem, gidx - 2)
                    off = 0
                    for table, ng, idx_sb, off8 in (
                        (full[L][0:min(VLO_, N), :], GLO[b], idxlo_sb, LO8[b]),
                        (full[L][VLO_:N, :], GHI[b], idxhi_sb, HI8[b]),
                    ):
                        done = 0
                        while done < ng:
                            k = min(GCAP, ng - done)
                            gpsimd.dma_gather(
                                msg_sb[:, slot * MAXG + off:slot * MAXG + off + k, :],
                                table,
                                idx_sb[:, off8 + done * 8:off8 + (done + k) * 8],
                                k * P, k * P, P,
                                queue_num=qctr[0] % NSWQ,
                            ).then_inc(g_sems[slot], 16)
                            qctr[0] += 1
                            done += k
                            off += k

        @block.scalar
        def _(scalar: bass.BassScalarEngine):
            scalar.wait_ge(init_sem, 1)
            scalar.activation(out=dinv_dest[:], in_=dinv_dest[:],
                              func=mybir.ActivationFunctionType.Sqrt)
            scalar.activation(out=dinv_node[:], in_=dinv_node[:],
                              func=mybir.ActivationFunctionType.Sqrt
                              ).then_inc(init_sem, 1)
            for L in range(NL):
                func = (mybir.ActivationFunctionType.Relu if L < NL - 1
                        else mybir.ActivationFunctionType.Identity)
                for b in range(B):
                    gidx = L * B + b
                    wb = wb_of(b)
                    bo = P * b
                    scalar.wait_ge(vepi_sem, gidx + 1)
                    scalar.activation(
                        out=hT_sb[:, bo:bo + wb],
                        in_=epi_tmp[:, gidx % 2, 0:wb],
                        func=func,
                        bias=bias_sb[:, L:L + 1],
                        scale=1.0,
                    ).then_inc(epi_sem, 1)

        @block.vector
        def _(vector: bass.BassVectorEngine):
            vector.wait_ge(in_sem, 16 * NIN)
            vector.wait_ge(pool_sem, -(-MAXG // GCAP))
            vector.tensor_copy(out=iota_bf[:], in_=iota_i[:])
            vector.reciprocal(out=dinv_dest[:], in_=dinv_dest[:])
            vector.reciprocal(out=dinv_node[:], in_=dinv_node[:]).then_inc(init_sem, 1)
            vector.wait_ge(init_sem, 2)

            def build_s(L, b):
                gidx = L * B + b
                if gidx >= 2:
                    vector.wait_ge(mm_sem, gidx - 1)
                g = GLO[b] + GHI[b]
                go = GOFF[b]
                vector.tensor_tensor(
                    out=s_sb[:, gidx % 2, 0:g, :],
                    in0=colrel_sb[:, go:go + g].to_broadcast([P, g, P]),
                    in1=iota_bf[:, 0:g, :],
                    op=mybir.AluOpType.is_equal,
                ).then_inc(s_sem, 1)

            def stage(Lp, b):
                sidx = Lp * B + b
                wb = wb_of(b)
                vector.wait_ge(gmm_sem, sidx + 1)
                if sidx >= 2:
                    vector.wait_ge(hsdma_sem, 16 * sidx)
                vector.tensor_scalar(
                    out=stage_sb[0:wb, sidx % 2, :],
                    in0=ps_g[sidx % 2][0:wb, 0:P],
                    scalar1=dinv_node[0:wb, b:b + 1],
                    scalar2=None,
                    op0=mybir.AluOpType.mult,
                ).then_inc(stage_sem, 1)

            for b in range(B):
                stage(0, b)
            for L in range(NL):
                build_s(L, 0)
                if B > 1:
                    build_s(L, 1)
                for b in range(B):
                    gidx = L * B + b
                    wb = wb_of(b)
                    bo = P * b
                    vector.wait_ge(mm_sem, gidx + 1)
                    if gidx >= 2:
                        vector.wait_ge(epi_sem, gidx - 1)
                    vector.tensor_tensor(
                        out=epi_tmp[:, gidx % 2, 0:wb],
                        in0=ps_a[gidx % 4][:, 0:wb],
                        in1=dinv_dest[:, bo:bo + wb],
                        op=mybir.AluOpType.mult,
                    ).then_inc(vepi_sem, 1)
                    if L < NL - 1:
                        stage(L + 1, b)
                    if b + 2 <= B - 1:
                        build_s(L, b + 2)

        @block.tensor
        def _(tensor: bass.BassTensorEngine):
            tensor.wait_ge(in_sem, 16 * NIN)

            def gemm(Lp, b, src_sb):
                sidx = Lp * B + b
                wb = wb_of(b)
                if Lp > 0:
                    tensor.wait_ge(epi_sem, (Lp - 1) * B + b + 1)
                if sidx >= 2:
                    tensor.wait_ge(stage_sem, sidx - 1)
                tensor.matmul(
                    out=ps_g[sidx % 2][0:wb, 0:P],
                    lhsT=src_sb[:, P * b:P * b + wb],
                    rhs=w_sb[:, Lp, :],
                    start=True, stop=True,
                ).then_inc(gmm_sem, 1)

            for b in range(B):
                gemm(0, b, xT_sb)
            for L in range(NL):
                for b in range(B):
                    gidx = L * B + b
                    slot = gidx % 3
                    tensor.wait_ge(g_sems[gidx % 3], g_cum[gidx])
                    tensor.wait_ge(s_sem, gidx + 1)
                    if gidx >= 4:
                        tensor.wait_ge(epi_sem, gidx - 3)
                    g_tot = GLO[b] + GHI[b]
                    for g in range(g_tot):
                        inst = tensor.matmul(
                            out=ps_a[gidx % 4][:, 0:P],
                            lhsT=msg_sb[:, slot * MAXG + g, :],
                            rhs=s_sb[:, gidx % 2, g, :],
                            start=(g == 0), stop=(g == g_tot - 1),
                        )
                    inst.then_inc(mm_sem, 1)
                    if L < NL - 1 and b >= 1:
                        gemm(L + 1, b - 1, hT_sb)
                if L < NL - 1:
                    gemm(L + 1, B - 1, hT_sb)

    nc.compile()
    return nc


# --------------------------------------------------------------------------
# entry point
# --------------------------------------------------------------------------

_CACHE = {}


def _get_compiled(edge_key, edge_index, n_nodes):
    if edge_key in _CACHE:
        return _CACHE[edge_key]
    sched, per_core = _host_prep(edge_index, n_nodes)
    nc = _build(sched)
    _CACHE[edge_key] = (sched, per_core, nc)
    return _CACHE[edge_key]


def run_device(x, edge_index, Ws, bs, trace=False):
    """Builds inputs, runs the SPMD kernel, returns (out [N,64], results)."""
    from concourse.bass_utils import run_bass_kernel_spmd

    n_nodes = x.shape[0]
    edge_key = hash(np.asarray(edge_index).tobytes()) ^ n_nodes
    sched, per_core, nc = _get_compiled(edge_key, edge_index, n_nodes)
    sh = sched["sh"]

    x = np.asarray(x, np.float32)
    xT = np.ascontiguousarray(x.T)
    W1, W2, W3 = (np.asarray(w, np.float32) for w in Ws)
    b1, b2, b3 = (np.asarray(b, np.float32).reshape(-1) for b in bs)
    W3p = np.zeros((P, P), np.float32)
    W3p[:, :W3.shape[1]] = W3
    b3p = np.zeros((P,), np.float32)
    b3p[:b3.shape[0]] = b3

    in_maps = []
    for c in range(NCORE):
        pc = per_core[c]
        in_maps.append({
            "xT": np.ascontiguousarray(xT[:, c * sh:(c + 1) * sh]),
            "deg_bcast": pc["deg_bcast"],
            "deg_node": pc["deg_node"],
            "idx_lo": pc["idx_lo"],
            "idx_hi": pc["idx_hi"],
            "colrel": pc["colrel"],
            "W1": W1, "W2": W2, "W3": W3p,
            "b1": np.ascontiguousarray(b1[:, None]),
            "b2": np.ascontiguousarray(b2[:, None]),
            "b3": np.ascontiguousarray(b3p[:, None]),
        })

    res = run_bass_kernel_spmd(nc, in_maps, core_ids=list(range(NCORE)),
                               trace=trace)
    outs = [res.results[c]["out"] for c in range(NCORE)]
    out = np.concatenate(outs, axis=1).T[:, :W3.shape[1]]
    return np.ascontiguousarray(out, dtype=np.float32), res


def kernel(x, edge_index, W1, b1, W2, b2, W3, b3):
    out, _ = run_device(x, edge_index, (W1, W2, W3), (b1, b2, b3))
    return out
